# revision 4
# baseline (speedup 1.0000x reference)
"""Trainium2 Bass kernel for nn_CrossAttention: two-stream (rgb/depth) cross
attention, B=8 batch elements data-parallel across 8 NeuronCores.

Per core (one batch element b):
  rgb = x[:1024], depth = x[1024:]
  qkv_m = rgb/depth @ W_m + b_m          (H=8 heads, D=64)
  rgb_out   = softmax(q_dep k_rgb^T / 8) v_rgb   -> out tokens    0..1023
  depth_out = softmax(q_rgb k_dep^T / 8) v_dep   -> out tokens 1024..2047
  out = concat @ W_proj + b_proj

Design (v2 — deep pipeline, bf16 matmul paths):
  - All matmul operands bf16 (1 cyc/row on PE, halves SBUF/DMA); psum f32.
    Host pre-casts x and pre-swizzles weights to the SBUF layouts.
  - Emission order interleaves qkv production, attention units and the
    final projection so the scalar-engine exp stream (the critical
    resource, ~1.1us per [128,1024] tile, 128 tiles) starts ~6us into
    the kernel and never stalls: scores psum tag is double-buffered and
    reserved for scores only; all other matmul groups use separate tags.
  - Attention unit (d, g, qh): head-pair g, query-half qh, direction d.
    scores^T = k^T q via row-group-paired K=64 matmuls (hh0 on PE rows
    0:64, hh1 on rows 64:128, concurrent).  exp is unnormalized
    (scores ~ N(0,1)); V is augmented with a ones column so attn@v also
    emits the softmax denominator Z (output row 64).
  - pv for hh0 is split kv 128 -> 2x64 row-group-paired matmuls into two
    psum banks (added at evac); hh1 runs plain K=128 (psum budget: 8
    banks = scores 4 + pv-pair 2 + pv-single 1 + late-group 1).
  - Z reciprocals batched per (d, g): [16,128] DVE reciprocal instead of
    32 tiny ones; normalization multiply at oT evac, off critical path.
"""
import numpy as np
import ml_dtypes

import concourse.bass as bass
import concourse.mybir as mybir
import concourse.tile as tile
from concourse import bacc
from concourse.bass_utils import run_bass_kernel_spmd
from concourse.bass_interp import get_hw_module

f32 = mybir.dt.float32
bf16 = mybir.dt.bfloat16
AF = mybir.ActivationFunctionType
Alu = mybir.AluOpType

N_CORES = 8
B, N, C = 8, 2048, 512
H, D, L = 8, 64, 1024
SCALE = float(D) ** -0.5

# x DMA / transpose order: m0 tokens 0:512, m1 0:512, m0 512:1024, ...
TI_ORDER = [0, 1, 2, 3, 8, 9, 10, 11, 4, 5, 6, 7, 12, 13, 14, 15]


def _emit(nc, tc, x, wqk, bqk_d, wv, bv_d, wp, bp_d, out):
    from concourse.masks import make_identity

    with (
        tc.tile_pool(name="main", bufs=1) as P,
        tc.tile_pool(name="ps", bufs=1, space="PSUM") as PS,
    ):
        # ---------------- persistent tensors + setup DMAs ----------------
        ident = P.tile([128, 128], bf16, name="ident")
        make_identity(nc, ident[:, :])

        wqk_t = []
        wv_t = []
        bqk_t = []
        bv_t = []
        for m in range(2):
            wt = P.tile([128, 4, 1024], bf16, name=f"wqk{m}")
            nc.gpsimd.dma_start(out=wt[:, :, :], in_=wqk[m])
            wqk_t.append(wt)
            vt = P.tile([128, 4, 512], bf16, name=f"wv{m}")
            nc.gpsimd.dma_start(out=vt[:, :, :], in_=wv[m])
            wv_t.append(vt)
            bq = P.tile([128, 8], f32, name=f"bqk{m}")
            nc.sync.dma_start(out=bq[:, :], in_=bqk_d[m])
            bqk_t.append(bq)
            bv1 = P.tile([1, 512], f32, name=f"bv1_{m}")
            nc.sync.dma_start(out=bv1[:, :], in_=bv_d[m])
            bvm = P.tile([128, 512], f32, name=f"bv{m}")
            nc.gpsimd.partition_broadcast(bvm[:, :], bv1[:, :])
            bv_t.append(bvm)
        wp_t = P.tile([128, 4, 512], bf16, name="wp")
        nc.gpsimd.dma_start(out=wp_t[:, :, :], in_=wp)
        bp1 = P.tile([1, 512], f32, name="bp1")
        nc.sync.dma_start(out=bp1[:, :], in_=bp_d)
        bp_t = P.tile([128, 512], f32, name="bp")
        nc.gpsimd.partition_broadcast(bp_t[:, :], bp1[:, :])

        # V augmented with ones column (emits softmax denominator Z)
        vaug = [P.tile([128, 8, 8, 65], bf16, name=f"vaug{m}") for m in range(2)]
        ones64 = P.tile([128, 64], bf16, name="ones64")
        nc.gpsimd.memset(ones64[:, :], 1.0)
        for m in range(2):
            nc.vector.tensor_copy(
                vaug[m][:, :, :, 64:65],
                ones64.rearrange("p (c h o) -> p c h o", c=8, h=8))

        # xT: feature-major x, built by PE transpose
        xT = P.tile([128, 4, 2048], bf16, name="xT")
        # q/k tiles: per (m, group g, half): [128 feats, 512 tokens]
        qT = [[[P.tile([128, 512], bf16, name=f"q{m}_{g}_{h}")
                for h in range(2)] for g in range(4)] for m in range(2)]
        kT = [[[P.tile([128, 512], bf16, name=f"k{m}_{g}_{h}")
                for h in range(2)] for g in range(4)] for m in range(2)]
        # attention output, d-major (= proj lhsT): [128 feats, g, 1024 tok]
        oT = [P.tile([128, 4, 1024], bf16, name=f"oT{d}") for d in range(2)]

        # ---------------- emission helpers ----------------
        def emit_T(ti):
            """transpose x[ti*128:(ti+1)*128, :] -> xT[:, :, tok-range]"""
            xsrc = P.tile([128, 512], bf16, name=f"xsrc{ti}", tag="xsrc",
                          bufs=3)
            nc.sync.dma_start(out=xsrc[:, :], in_=x[ti * 128:(ti + 1) * 128, :])
            tp = PS.tile([128, 512], bf16, name=f"tp{ti}", tag="sc", bufs=2)
            for k in range(4):
                nc.tensor.transpose(tp[:, k * 128:(k + 1) * 128],
                                    xsrc[:, k * 128:(k + 1) * 128],
                                    ident[:, :])
            nc.vector.tensor_copy(
                xT[:, :, ti * 128:(ti + 1) * 128],
                tp.rearrange("p (k t) -> p k t", k=4))

        def emit_qk(m, jt, g, h, late):
            """produce qT/kT[m][g][h]: jt=0 for q, 1 for k; h = token half"""
            j = jt * 4 + g
            dst = (kT if jt else qT)[m][g][h]
            tag, bufs = ("late", 1) if late else ("sc", 2)
            ps = PS.tile([128, 512], f32, name=f"qk{m}_{j}_{h}", tag=tag,
                         bufs=bufs)
            t0 = m * 1024 + h * 512
            for k in range(4):
                nc.tensor.matmul(
                    ps[:, :],
                    wqk_t[m][:, k, j * 128:(j + 1) * 128],
                    xT[:, k, t0:t0 + 512],
                    start=(k == 0), stop=(k == 3))
            nc.vector.tensor_scalar(
                out=dst[:, :], in0=ps[:, :],
                scalar1=bqk_t[m][:, j:j + 1], scalar2=None, op0=Alu.add)

        def emit_v(m, c, late):
            tag, bufs = ("late", 1) if late else ("sc", 2)
            ps = PS.tile([128, 512], f32, name=f"v{m}_{c}", tag=tag, bufs=bufs)
            for k in range(4):
                nc.tensor.matmul(
                    ps[:, :],
                    xT[:, k, m * 1024 + c * 128:m * 1024 + (c + 1) * 128],
                    wv_t[m][:, k, :],
                    start=(k == 0), stop=(k == 3))
            nc.vector.tensor_tensor(
                out=vaug[m][:, c, :, 0:64],
                in0=ps.rearrange("p (h d) -> p h d", h=8),
                in1=bv_t[m].rearrange("p (h d) -> p h d", h=8),
                op=Alu.add)

        # per-(d,g) Z batch state: list of (oTs tile, uh index, qh, hh)
        zgroup = {}

        def emit_unit(d, g, qh):
            """attention unit: scores+exp+pv for (direction d, pair g, qh)"""
            qm, kvm = 1 - d, d
            # pv accumulators: hh0 row-paired (2 banks), hh1 plain (1 bank)
            pv0a = PS.tile([65, 512], f32, name=f"o{d}{g}{qh}a", tag="pv2",
                           bufs=2)
            pv0b = PS.tile([65, 512], f32, name=f"o{d}{g}{qh}b", tag="pv2",
                           bufs=2)
            pv1 = PS.tile([65, 512], f32, name=f"o{d}{g}{qh}c", tag="pv1",
                          bufs=1)

            def emit_scores(c):
                s_ps = PS.tile([128, 1024], f32, name=f"s{d}{g}{qh}{c}",
                               tag="sc", bufs=2)
                for hh in range(2):
                    pb = hh * 64
                    nc.tensor.matmul(
                        s_ps[:, hh * 512:(hh + 1) * 512],
                        kT[kvm][g][c // 4][pb:pb + 64,
                                           (c % 4) * 128:(c % 4 + 1) * 128],
                        qT[qm][g][qh][pb:pb + 64, :],
                        start=True, stop=True)
                exp_t = P.tile([128, 1024], bf16, name=f"e{d}{g}{qh}{c}",
                               tag="exp", bufs=12)
                nc.scalar.activation(exp_t[:, :], s_ps[:, :], AF.Exp,
                                     scale=SCALE)
                return exp_t

            def emit_pv(c, exp_t):
                st = dict(start=(c == 0), stop=(c == 7))
                nc.tensor.matmul(pv0a[:, :],
                                 vaug[kvm][0:64, c, 2 * g, :],
                                 exp_t[0:64, 0:512], **st)
                nc.tensor.matmul(pv0b[:, :],
                                 vaug[kvm][64:128, c, 2 * g, :],
                                 exp_t[64:128, 0:512], **st)
                nc.tensor.matmul(pv1[:, :],
                                 vaug[kvm][:, c, 2 * g + 1, :],
                                 exp_t[:, 512:1024], **st)

            exps = {0: emit_scores(0)}
            for c in range(1, 8):
                exps[c] = emit_scores(c)
                emit_pv(c - 1, exps.pop(c - 1))
            emit_pv(7, exps.pop(7))

            # evac: hh0 = pair-add, hh1 = copy; Z rides along as row 64
            for hh in range(2):
                oTs = P.tile([65, 512], f32, name=f"oTs{d}{g}{qh}{hh}",
                             tag="oTs", bufs=8)
                if hh == 0:
                    # TT with two PSUM operands is rejected by the BIR
                    # verifier: evacuate one half first, add the other.
                    nc.vector.tensor_copy(oTs[:, :], pv0a[:, :])
                    nc.vector.tensor_tensor(out=oTs[:, :], in0=oTs[:, :],
                                            in1=pv0b[:, :], op=Alu.add)
                else:
                    nc.vector.tensor_copy(oTs[:, :], pv1[:, :])
                zgroup.setdefault((d, g), []).append((oTs, qh, hh))

        def emit_zgroup(d, g):
            """batched 1/Z + normalization for the 4 unit-halves of (d,g)"""
            entries = zgroup.pop((d, g))
            assert len(entries) == 4
            zg = P.tile([16, 128], f32, name=f"zg{d}{g}", tag="zg", bufs=2)
            for i, (oTs, qh, hh) in enumerate(entries):
                nc.sync.dma_start(
                    out=zg[4 * i:4 * i + 4, :],
                    in_=oTs[64:65, :].rearrange("o (j t) -> o j t", j=4))
            rzg = P.tile([16, 128], f32, name=f"rzg{d}{g}", tag="rzg", bufs=2)
            nc.vector.reciprocal(rzg[:, :], zg[:, :])
            for i, (oTs, qh, hh) in enumerate(entries):
                rz = P.tile([1, 512], f32, name=f"rz{d}{g}{i}", tag="rz",
                            bufs=4)
                nc.sync.dma_start(
                    out=rz[0:1, :].rearrange("o (j t) -> o j t", j=4),
                    in_=rzg[4 * i:4 * i + 4, :])
                rzb = P.tile([64, 512], f32, name=f"rzb{d}{g}{i}", tag="rzb",
                             bufs=4)
                nc.gpsimd.partition_broadcast(rzb[:, :], rz[:, :])
                nc.vector.tensor_tensor(
                    out=oT[d][hh * 64:hh * 64 + 64, g,
                              qh * 512:(qh + 1) * 512],
                    in0=oTs[0:64, :], in1=rzb[:, :], op=Alu.mult)

        def emit_pj(d, tt):
            ps = PS.tile([128, 512], f32, name=f"pj{d}_{tt}", tag="late",
                         bufs=1)
            for g in range(4):
                nc.tensor.matmul(
                    ps[:, :],
                    oT[d][:, g, tt * 128:(tt + 1) * 128],
                    wp_t[:, g, :],
                    start=(g == 0), stop=(g == 3))
            ost = P.tile([128, 512], f32, name=f"ost{d}_{tt}", tag="ost",
                         bufs=3)
            nc.vector.tensor_tensor(out=ost[:, :], in0=ps[:, :],
                                    in1=bp_t[:, :], op=Alu.add)
            nc.sync.dma_start(
                out=out[d * 1024 + tt * 128:d * 1024 + (tt + 1) * 128, :],
                in_=ost[:, :])

        # ---------------- schedule ----------------
        # warmup: transposes + the qk groups feeding the first units
        for ti in TI_ORDER[0:4]:
            emit_T(ti)
        emit_qk(0, 1, 0, 0, late=False)          # kT[0][g0] tok 0:512
        for ti in TI_ORDER[4:8]:
            emit_T(ti)
        emit_qk(1, 0, 0, 0, late=False)          # qT[1][g0] qh0
        for ti in TI_ORDER[8:12]:
            emit_T(ti)
        emit_qk(0, 1, 0, 1, late=False)          # kT[0][g0] tok 512:1024
        for ti in TI_ORDER[12:16]:
            emit_T(ti)
        for c in range(8):                       # vaug[0]
            emit_v(0, c, late=False)
        emit_qk(1, 0, 0, 1, late=False)          # qT[1][g0] qh1

        emit_unit(0, 0, 0)
        # interleave the remaining qkv groups among the d0 units
        emit_qk(0, 1, 1, 0, late=True)
        emit_qk(0, 1, 1, 1, late=True)
        emit_qk(1, 0, 1, 0, late=True)
        emit_unit(0, 0, 1)
        emit_zgroup(0, 0)
        emit_qk(1, 0, 1, 1, late=True)
        emit_qk(0, 1, 2, 0, late=True)
        emit_qk(0, 1, 2, 1, late=True)
        emit_unit(0, 1, 0)
        emit_qk(1, 0, 2, 0, late=True)
        emit_qk(1, 0, 2, 1, late=True)
        emit_qk(0, 1, 3, 0, late=True)
        emit_unit(0, 1, 1)
        emit_zgroup(0, 1)
        emit_qk(0, 1, 3, 1, late=True)
        emit_qk(1, 0, 3, 0, late=True)
        emit_qk(1, 0, 3, 1, late=True)
        emit_unit(0, 2, 0)
        emit_qk(1, 1, 0, 0, late=True)           # kT[1][g0]
        emit_qk(1, 1, 0, 1, late=True)
        emit_qk(0, 0, 0, 0, late=True)           # qT[0][g0]
        emit_unit(0, 2, 1)
        emit_zgroup(0, 2)
        for c in range(4):                       # vaug[1] first half
            emit_v(1, c, late=True)
        emit_unit(0, 3, 0)
        for c in range(4, 8):
            emit_v(1, c, late=True)
        emit_qk(0, 0, 0, 1, late=True)
        emit_unit(0, 3, 1)
        emit_zgroup(0, 3)
        emit_qk(1, 1, 1, 0, late=True)
        emit_qk(1, 1, 1, 1, late=True)
        emit_qk(0, 0, 1, 0, late=True)
        emit_qk(0, 0, 1, 1, late=True)

        emit_unit(1, 0, 0)
        emit_qk(1, 1, 2, 0, late=True)
        emit_qk(1, 1, 2, 1, late=True)
        emit_qk(0, 0, 2, 0, late=True)
        emit_unit(1, 0, 1)
        emit_zgroup(1, 0)
        emit_qk(0, 0, 2, 1, late=True)
        emit_qk(1, 1, 3, 0, late=True)
        emit_qk(1, 1, 3, 1, late=True)
        emit_unit(1, 1, 0)
        emit_qk(0, 0, 3, 0, late=True)
        emit_qk(0, 0, 3, 1, late=True)
        emit_unit(1, 1, 1)
        emit_zgroup(1, 1)
        emit_pj(0, 0)
        emit_pj(0, 1)
        emit_unit(1, 2, 0)
        emit_pj(0, 2)
        emit_pj(0, 3)
        emit_unit(1, 2, 1)
        emit_zgroup(1, 2)
        emit_pj(0, 4)
        emit_pj(0, 5)
        emit_unit(1, 3, 0)
        emit_pj(0, 6)
        emit_pj(0, 7)
        emit_unit(1, 3, 1)
        emit_zgroup(1, 3)
        for tt in range(8):
            emit_pj(1, tt)


def build_module():
    nc = bacc.Bacc("TRN2", target_bir_lowering=False, debug=False,
                   num_devices=N_CORES)
    x = nc.dram_tensor("x", [N, C], bf16, kind="ExternalInput").ap()
    wqk = [nc.dram_tensor(f"wqk{m}", [128, 4, 1024], bf16,
                          kind="ExternalInput").ap() for m in range(2)]
    bqk = [nc.dram_tensor(f"bqk{m}", [128, 8], f32,
                          kind="ExternalInput").ap() for m in range(2)]
    wv = [nc.dram_tensor(f"wv{m}", [128, 4, 512], bf16,
                         kind="ExternalInput").ap() for m in range(2)]
    bv = [nc.dram_tensor(f"bv{m}", [1, 512], f32,
                         kind="ExternalInput").ap() for m in range(2)]
    wp = nc.dram_tensor("wp", [128, 4, 512], bf16, kind="ExternalInput").ap()
    bp = nc.dram_tensor("bp", [1, 512], f32, kind="ExternalInput").ap()
    out = nc.dram_tensor("out", [N, C], f32, kind="ExternalOutput").ap()

    with tile.TileContext(nc) as tc:
        _emit(nc, tc, x, wqk, bqk, wv, bv, wp, bp, out)
    nc.compile()
    nc.m = get_hw_module(nc.m)
    return nc


_NC_CACHE = None


def _prep_shared(W_rgb_qkv, b_rgb_qkv, W_depth_qkv, b_depth_qkv,
                 W_proj, b_proj):
    """Host-side weight swizzle/cast to the kernel's SBUF layouts."""
    bf = ml_dtypes.bfloat16
    shared = {}
    for m, (W, b) in enumerate([(W_rgb_qkv, b_rgb_qkv),
                                (W_depth_qkv, b_depth_qkv)]):
        W = np.asarray(W, np.float32)
        b = np.asarray(b, np.float32)
        shared[f"wqk{m}"] = np.ascontiguousarray(
            W[:, 0:1024].reshape(4, 128, 1024).transpose(1, 0, 2)).astype(bf)
        shared[f"wv{m}"] = np.ascontiguousarray(
            W[:, 1024:1536].reshape(4, 128, 512).transpose(1, 0, 2)).astype(bf)
        shared[f"bqk{m}"] = np.ascontiguousarray(
            b[0:1024].reshape(8, 128).T).astype(np.float32)
        shared[f"bv{m}"] = np.ascontiguousarray(
            b[1024:1536].reshape(1, 512)).astype(np.float32)
    Wp = np.asarray(W_proj, np.float32)
    shared["wp"] = np.ascontiguousarray(
        Wp.reshape(4, 128, 512).transpose(1, 0, 2)).astype(bf)
    shared["bp"] = np.ascontiguousarray(
        np.asarray(b_proj, np.float32).reshape(1, 512))
    return shared


def kernel(x, W_rgb_qkv, b_rgb_qkv, W_depth_qkv, b_depth_qkv, W_proj, b_proj):
    global _NC_CACHE
    if _NC_CACHE is None:
        _NC_CACHE = build_module()
    nc = _NC_CACHE

    xb = np.asarray(x, np.float32).astype(ml_dtypes.bfloat16)
    shared = _prep_shared(W_rgb_qkv, b_rgb_qkv, W_depth_qkv, b_depth_qkv,
                          W_proj, b_proj)
    in_maps = [{"x": np.ascontiguousarray(xb[i]), **shared}
               for i in range(N_CORES)]
    res = run_bass_kernel_spmd(nc, in_maps, core_ids=list(range(N_CORES)))
    return np.stack([res.results[i]["out"] for i in range(N_CORES)], axis=0)
